# revision 11
# baseline (speedup 1.0000x reference)
"""Multi-head attention (B=2, S=2048, D=1024, H=16) on 8 Trainium2 NeuronCores.

Sharding: core c -> batch b = c // 4, head-group g = c % 4 (4 heads = 256 proj
dims per core). Each core computes its 4 heads' attention plus the matching
slice of the output projection; the host sums the 4 partial outputs per batch
and adds bo.

Device layouts (matmul operands fp16 = IEEE half at bf16 PE rate):
  qT/kT [o, s]   : proj from host-transposed Q/K (contraction on partitions)
  v     [s, o]   : natural layout + ones column per head (softmax denominator
                   rides along row 64 of the PV matmul output)
  scoresT [k, q] : head pairs row-packed on the PE (base_partition 0/64);
                   both halves of a [128,1024] PSUM tile -> one wide Exp
  outT  [d, q]   : unnormalized; moved off PSUM fast, normalized with
                   reciprocal_approx_fast + GpSimd partition_broadcast
  out_pT [o, q]  : local slice of x @ Wo.T; host transposes + sums + bias

Schedule: the exp evacuation is the critical resource (ScalarE is 1
elem/cycle/lane at 1.2 GHz -> ~1.15us per 128x1024 chunk), so
  - 5 of 16 key chunks per phase compute exp on the DVE instead, via the
    Schraudolph bit trick (int16 = round(s*scale*log2e*1024 + bias), bitcast
    to fp16 = 2^y with <3.1% elementwise error; numerator and denominator
    use the same approximation so softmax error mostly cancels);
  - inputs land via a few wide DMA triggers on both HWDGE rings, ordered so
    kT arrives first; all projection chains except the first two run inside
    the qb0 key loop, under the DMA shadow, with DVE (not ACT) evacuations;
  - qb0 pair0's PV lags 8 chunks so v-chains never block the PE queue while
    vT streams in; the next query block's qT chain is spread one matmul per
    key chunk across pair 1; output-projection matmuls ride in pair 0.
"""

import ml_dtypes
import numpy as np

import concourse.bass as bass
import concourse.mybir as mybir
import concourse.tile as tile
from concourse import bacc
from concourse.bass_utils import run_bass_kernel_spmd

B, S, D, H = 2, 2048, 1024, 16
OL = 256          # local projection dims (4 heads x 64)
NI = D // 128     # contraction chunks for projections
NK = S // 128     # key chunks
NQ = S // 512     # query blocks

# bit-trick exp: fp16 = bitcast(int16(round(x*EXP_A + EXP_B))) ~= 2^(x*lg2e/8)
EXP_A = 1024.0 * 1.4426950408889634 * 0.125
EXP_B = 15360.0 - 44.0
DVE_KCS = (2, 6, 10, 14)   # key chunks whose exp runs on the DVE

_CACHE = {}


def _build():
    DT = mybir.dt.float16
    F32 = mybir.dt.float32
    I16 = mybir.dt.int16
    AF = mybir.ActivationFunctionType
    ALU = mybir.AluOpType

    nc = bacc.Bacc("TRN2", target_bir_lowering=False, debug=False, num_devices=8)

    qt_d = nc.dram_tensor("qt", [D, S], DT, kind="ExternalInput").ap() \
        .rearrange("(c p) s -> p c s", p=128)
    kt_d = nc.dram_tensor("kt", [D, S], DT, kind="ExternalInput").ap() \
        .rearrange("(c p) s -> p c s", p=128)
    vt_d = nc.dram_tensor("vt", [D, S], DT, kind="ExternalInput").ap() \
        .rearrange("(c p) s -> p c s", p=128)
    wq_d = nc.dram_tensor("wqt", [D, OL], DT, kind="ExternalInput").ap() \
        .rearrange("(c p) o -> p c o", p=128)
    wk_d = nc.dram_tensor("wkt", [D, OL], DT, kind="ExternalInput").ap() \
        .rearrange("(c p) o -> p c o", p=128)
    wv_d = nc.dram_tensor("wvt", [D, OL], DT, kind="ExternalInput").ap() \
        .rearrange("(c p) o -> p c o", p=128)
    bq_d = nc.dram_tensor("bq2", [2, 128, 1], F32, kind="ExternalInput").ap()
    bk_d = nc.dram_tensor("bk2", [2, 128, 1], F32, kind="ExternalInput").ap()
    bv_d = nc.dram_tensor("bv1", [1, OL], DT, kind="ExternalInput").ap()
    wo_d = nc.dram_tensor("wot", [OL, D], DT, kind="ExternalInput").ap() \
        .rearrange("(c p) o -> c p o", p=128)
    out_d = nc.dram_tensor("out_t", [D, S], F32, kind="ExternalOutput").ap() \
        .rearrange("(c p) s -> c p s", p=128)

    with tile.TileContext(nc) as tc:
        with (
            tc.tile_pool(name="per", bufs=1) as per,
            tc.tile_pool(name="wp", bufs=1) as wp,
            tc.tile_pool(name="ip", bufs=1) as ip,
            tc.tile_pool(name="pr", bufs=11) as pr,
            tc.tile_pool(name="sm", bufs=2) as sm,
            tc.tile_pool(name="ot", bufs=2) as ot,
            tc.tile_pool(name="osg", bufs=3) as osg,
            tc.tile_pool(name="pj", bufs=2, space="PSUM") as pj,
            tc.tile_pool(name="p1", bufs=2, space="PSUM") as p1,
            tc.tile_pool(name="px", bufs=2, space="PSUM") as px,
        ):
            # --- persistent tiles
            qt_sb = [per.tile([128, S], DT, tag=f"qt{m}", name=f"qt{m}")
                     for m in range(2)]
            kt_sb = [per.tile([128, S], DT, tag=f"kt{m}", name=f"kt{m}")
                     for m in range(2)]
            v_sb = [per.tile([128, 4, 65], DT, tag=f"v{sc}", name=f"v{sc}")
                    for sc in range(NK)]
            wo_sb = [per.tile([128, D], DT, tag=f"wo{c}", name=f"wo{c}")
                     for c in range(2)]
            bq_sb = [per.tile([128, 1], F32, tag=f"bq{m}", name=f"bq{m}")
                     for m in range(2)]
            bk_sb = [per.tile([128, 1], F32, tag=f"bk{m}", name=f"bk{m}")
                     for m in range(2)]
            bv_sb = per.tile([1, OL], DT, tag="bv", name="bv")
            ones_f = per.tile([1, 128], F32, tag="ones_f", name="ones_f")
            vones_f = per.tile([128, 1], F32, tag="vones_f", name="vones_f")
            ones_r = per.tile([1, 128], DT, tag="ones_r", name="ones_r")
            nc.vector.memset(ones_f[:], 1.0)
            nc.vector.memset(vones_f[:], 1.0)
            nc.vector.tensor_copy(ones_r[:], ones_f[:])

            # --- mega input tiles: one [128, NI, *] tile per tensor so a
            # single DMA trigger moves a whole column range of all NI
            # contraction chunks across all 16 SDMA engines.
            as_k = ip.tile([128, NI, S], DT, tag="ak", name="ak")
            as_q = ip.tile([128, NI, S], DT, tag="aq", name="aq")
            as_v = ip.tile([128, NI, S], DT, tag="av", name="av")
            ws_k = wp.tile([128, NI, OL], DT, tag="wk", name="wk")
            ws_q = wp.tile([128, NI, OL], DT, tag="wq", name="wq")
            ws_v = wp.tile([128, NI, OL], DT, tag="wv", name="wv")

            # scalar ring: biases + weights + qT rest; sync ring: kT q0,
            # qT s0, then kT/vT quarters interleaved at consumption pace.
            for m in range(2):
                nc.scalar.dma_start(bk_sb[m][:], bk_d[m])
            for m in range(2):
                nc.scalar.dma_start(bq_sb[m][:], bq_d[m])
            nc.scalar.dma_start(ws_k[:], wk_d)
            nc.scalar.dma_start(ws_q[:], wq_d)
            nc.scalar.dma_start(bv_sb[:], bv_d)
            nc.scalar.dma_start(ws_v[:], wv_d)
            for c in range(2):
                nc.scalar.dma_start(wo_sb[c][:], wo_d[c])
            nc.scalar.dma_start(as_q[:, :, 512:2048], qt_d[:, :, 512:2048])

            def quarter(dst, src, j):
                csl = slice(j * 512, (j + 1) * 512)
                nc.sync.dma_start(dst[:, :, csl], src[:, :, csl])

            quarter(as_k, kt_d, 0)
            nc.sync.dma_start(as_q[:, :, 0:512], qt_d[:, :, 0:512])
            quarter(as_k, kt_d, 1)
            quarter(as_v, vt_d, 0)
            quarter(as_k, kt_d, 2)
            quarter(as_v, vt_d, 1)
            quarter(as_k, kt_d, 3)
            quarter(as_v, vt_d, 2)
            quarter(as_v, vt_d, 3)

            # warm the PE (HAM un-throttles after ~3.4us of activity) and
            # preload the exp table while the first DMAs land.
            junk = pj.tile([128, 64], F32, tag="pj", name="junk")
            warm = per.tile([1, 64], DT, tag="warm", name="warm")
            nc.vector.tensor_copy(warm[:], ones_f[:, 0:64])
            nc.scalar.activation(warm[:], ones_f[:, 0:64], AF.Exp)
            for _ in range(120):
                nc.tensor.matmul(junk[:], ones_r[:], warm[:],
                                 start=True, stop=True)

            def q_chain(ws, as_, bias_sb, dst_sb, m, s, on_act,
                        i0=0, i1=NI, acc=None):
                if acc is None:
                    acc = pj.tile([128, 512], F32, tag="pj", name="pj")
                for i in range(i0, i1):
                    nc.tensor.matmul(
                        acc[:],
                        ws[:, i, m * 128:(m + 1) * 128],
                        as_[:, i, s * 512:(s + 1) * 512],
                        start=(i == 0),
                        stop=(i == NI - 1),
                    )
                if i1 < NI:
                    return acc
                dst = dst_sb[m][:, s * 512:(s + 1) * 512]
                if on_act:
                    nc.scalar.activation(
                        dst, acc[:], AF.Identity, bias=bias_sb[m][:])
                else:
                    nc.vector.tensor_scalar_add(dst, acc[:], bias_sb[m][:])
                return None

            def v_chain(sc):
                acc = pj.tile([128, OL], F32, tag="pj", name="pj")
                for i in range(NI):
                    nc.tensor.matmul(
                        acc[:],
                        as_v[:, i, sc * 128:(sc + 1) * 128],
                        ws_v[:, i, :],
                        start=(i == 0),
                        stop=False,
                    )
                nc.tensor.matmul(
                    acc[:], ones_r[:], bv_sb[:], start=False, stop=True
                )
                for h in range(4):
                    nc.vector.tensor_copy(
                        v_sb[sc][:, h, 0:64],
                        acc[:, h * 64:(h + 1) * 64],
                    )

            for sc in range(NK):
                nc.vector.tensor_copy(
                    v_sb[sc][:, :, 64:65],
                    vones_f[:].to_broadcast((128, 4, 1)),
                )
            # prefix: just enough to start the exp pipeline — kT[sg0] and
            # qT[s0] for the first head pair (ACT is idle here).
            q_chain(ws_k, as_k, bk_sb, kt_sb, 0, 0, True)
            q_chain(ws_q, as_q, bq_sb, qt_sb, 0, 0, True)

            # remaining chains run as full bursts inside qb0 pair0, one
            # chain per kc slot, under the DMA shadow; DVE does the bias
            # evacuation so ACT stays dedicated to exp.  (m0, sg) is ready
            # by kc=4*sg; (m1, *) + qT[s0, m1] by pair 1.
            chain_sched = {
                0: ("k", 0, 1), 1: ("k", 1, 0), 2: ("k", 0, 2),
                3: ("k", 1, 1), 4: ("k", 0, 3), 5: ("k", 1, 2),
                6: ("k", 1, 3), 7: ("q", 1, 0),
            }

            # --- attention + output projection, per query block
            def emit_op(qb, ots_prev, oc, pool, tg):
                osl = slice(oc * 128, (oc + 1) * 128)
                pso = pool.tile([128, 512], F32, tag=tg, name="pso")
                for c in range(2):
                    nc.tensor.matmul(
                        pso[:], wo_sb[c][:, osl], ots_prev[c][:],
                        start=(c == 0), stop=(c == 1),
                    )
                st = osg.tile([128, 512], F32, tag="st", name="st")
                nc.vector.tensor_copy(st[:], pso[:])
                nc.sync.dma_start(
                    out_d[oc][:, qb * 512:(qb + 1) * 512], st[:])

            def pv(acc, pair, pkc, pprob):
                for hh in range(2):
                    nc.tensor.matmul(
                        acc[hh][:], v_sb[pkc][:, pair * 2 + hh, :],
                        pprob[:, hh * 512:(hh + 1) * 512],
                        start=(pkc == 0), stop=(pkc == NK - 1),
                    )

            ots_prev = None
            for qb in range(NQ):
                qsl = slice(qb * 512, (qb + 1) * 512)
                ots = [ot.tile([128, 512], DT, tag=f"c{c}", name=f"otc{c}")
                       for c in range(2)]
                for pair in range(2):
                    first = qb == 0 and pair == 0
                    pend_max = 6 if first else 2
                    acc = [px.tile([65, 512], F32, tag="x", name="acc")
                           for _ in range(2)]
                    pend = []
                    op_iter = None
                    if pair == 0 and ots_prev is not None:
                        op_iter = iter(range(8))
                    if pair == 1 and qb < NQ - 1:
                        qaccs = [pj.tile([128, 512], F32, tag="pj",
                                         name="qacc")
                                 for _ in range(2)]
                    for kc in range(NK):
                        if first:
                            # stream the remaining projection chains under
                            # the DMA shadow
                            if kc in chain_sched:
                                kind, m, sg = chain_sched[kc]
                                ws, as_, bias, dst = (
                                    (ws_k, as_k, bk_sb, kt_sb) if kind == "k"
                                    else (ws_q, as_q, bq_sb, qt_sb))
                                q_chain(ws, as_, bias, dst, m, sg, False)
                            if kc >= 5:
                                v_chain(kc - 5)
                        if pair == 1 and qb < NQ - 1:
                            # stream qT[s=qb+1] projection: one contraction
                            # matmul per kc (i = kc//2, m = kc%2)
                            sN = qb + 1
                            i, m = kc // 2, kc % 2
                            nc.tensor.matmul(
                                qaccs[m][:],
                                ws_q[:, i, m * 128:(m + 1) * 128],
                                as_q[:, i, sN * 512:(sN + 1) * 512],
                                start=(i == 0),
                                stop=(i == NI - 1),
                            )
                            if kc >= 14:
                                # ACT (which has slack) evacuates the chain
                                # m=0 at kc14 so the next qb never waits on
                                # the loaded DVE queue
                                me = kc - 14
                                nc.scalar.activation(
                                    qt_sb[me][:, sN * 512:(sN + 1) * 512],
                                    qaccs[me][:], AF.Identity,
                                    bias=bq_sb[me][:])
                        ksl = slice(kc * 128, (kc + 1) * 128)
                        ps1 = p1.tile([128, 1024], F32, tag="s", name="s")
                        for hh in range(2):
                            psl = slice(hh * 64, (hh + 1) * 64)
                            nc.tensor.matmul(
                                ps1[:, hh * 512:(hh + 1) * 512],
                                kt_sb[pair][psl, ksl],
                                qt_sb[pair][psl, qsl],
                                start=True, stop=True,
                            )
                        prob = pr.tile([128, 1024], DT, tag="p", name="p")
                        if not first and kc in DVE_KCS:
                            # 2^x bit trick on the DVE to offload ACT
                            nc.vector.tensor_scalar(
                                prob[:].bitcast(I16), ps1[:], EXP_A, EXP_B,
                                ALU.mult, ALU.add,
                            )
                        else:
                            nc.scalar.activation(
                                prob[:], ps1[:], AF.Exp, scale=0.125
                            )
                        pend.append((kc, prob))
                        if len(pend) > pend_max:
                            pkc, pprob = pend.pop(0)
                            pv(acc, pair, pkc, pprob)
                        if op_iter is not None and kc % 2 == 1:
                            oc = next(op_iter, None)
                            if oc is not None:
                                emit_op(qb - 1, ots_prev, oc, pj, "pj")
                    if first:
                        # drain: interleave late v-chains with lagging PV
                        for j in range(11, NK):
                            v_chain(j)
                            pkc, pprob = pend.pop(0)
                            pv(acc, pair, pkc, pprob)
                    for pkc, pprob in pend:
                        pv(acc, pair, pkc, pprob)
                    # normalize off-bank: free both acc banks first
                    uns, dens = [], []
                    for hh in range(2):
                        un = sm.tile([64, 512], F32, tag=f"un{hh}",
                                     name=f"un{hh}")
                        nc.vector.tensor_copy(un[:], acc[hh][0:64, :])
                        den = sm.tile([1, 512], F32, tag=f"den{hh}",
                                      name=f"den{hh}")
                        nc.vector.tensor_copy(den[:], acc[hh][64:65, :])
                        uns.append(un)
                        dens.append(den)
                    for hh in range(2):
                        rec = sm.tile([1, 512], F32, tag="rec", name="rec")
                        nc.vector.reciprocal_approx_fast(rec[:], dens[hh][:])
                        rb = sm.tile([64, 512], F32, tag="rb", name="rb")
                        nc.gpsimd.partition_broadcast(rb[:], rec[:])
                        nc.vector.tensor_mul(
                            ots[pair][hh * 64:(hh + 1) * 64, :],
                            uns[hh][:], rb[:],
                        )
                ots_prev = ots
            for oc in range(8):
                emit_op(NQ - 1, ots_prev, oc, (pj, px)[oc % 2],
                        ("pj", "x")[oc % 2])

    nc.compile()
    return nc


def _get_nc():
    if "nc" not in _CACHE:
        _CACHE["nc"] = _build()
    return _CACHE["nc"]


def kernel(Q, K, V, Wq, bq, Wk, bk, Wv, bv, Wo, bo):
    nc = _get_nc()
    f = np.float32
    bf = np.float16
    in_maps = []
    for core in range(8):
        b, g = divmod(core, 4)
        sl = slice(g * OL, (g + 1) * OL)
        in_maps.append({
            "qt": np.ascontiguousarray(Q[b].T, dtype=bf),
            "kt": np.ascontiguousarray(K[b].T, dtype=bf),
            "vt": np.ascontiguousarray(V[b].T, dtype=bf),
            "wqt": np.ascontiguousarray(Wq[sl].T, dtype=bf),
            "wkt": np.ascontiguousarray(Wk[sl].T, dtype=bf),
            "wvt": np.ascontiguousarray(Wv[sl].T, dtype=bf),
            "bq2": np.ascontiguousarray(bq[sl].reshape(2, 128, 1), dtype=f),
            "bk2": np.ascontiguousarray(bk[sl].reshape(2, 128, 1), dtype=f),
            "bv1": np.ascontiguousarray(bv[sl].reshape(1, OL), dtype=bf),
            "wot": np.ascontiguousarray(Wo[:, sl].T, dtype=bf),
        })
    res = run_bass_kernel_spmd(nc, in_maps, core_ids=list(range(8)))
    out = np.empty((B, S, D), np.float32)
    for b in range(B):
        acc = res.results[b * 4 + 0]["out_t"].astype(np.float64)
        for g in range(1, 4):
            acc += res.results[b * 4 + g]["out_t"]
        out[b] = (acc.T + bo).astype(np.float32)
    return out
